# revision 2
# baseline (speedup 1.0000x reference)
"""CharRNN (B=128, S=256, V=96, H=1024, L=3) on 8 TRN2 NeuronCores.

Self-contained kernel: data-parallel over batch (16 per core), full fp32.
See build_charrnn for the per-core Bass/Tile program. Host side shards the
batch, transposes weights into feature-major layout, runs the SPMD program
on cores 0-7, and reassembles the full (logprobs, hidden) outputs.
"""
from contextlib import ExitStack

import numpy as np

import concourse.bass as bass
import concourse.tile as tile
from concourse import bacc, mybir
from concourse.bass_utils import run_bass_kernel_spmd

F32 = mybir.dt.float32
AF = mybir.ActivationFunctionType
ALU = mybir.AluOpType
P = 128

# problem config (hardcoded per spec)
N_CORES = 8
B_FULL, S, V, H, L = 128, 256, 96, 1024, 3
B_LOC = B_FULL // N_CORES
T_C = 8
KC = H // P


def build_charrnn(ctx: ExitStack, tc, outs: dict, ins: dict, cfg: dict):
    """Emit the per-core CharRNN program.

    Layouts (all f32, feature-major so the recurrence needs no transposes):
      h buffers   [128(p=k%128), KC, T_c, B]  (h[b, kc*128+p] at local t)
      pre buffers [128(p=j%128), JC, T_c, B]
      weights     W^T tiles [128(k), kc, j]   (lhsT stationary)

    Per rec step: 64 matmuls (jm x kc), W-stationary, N=B; psum [128, JC, B]
    accumulates; DVE adds the pre slice; ACT tanh writes the h slice. The 3
    layer recurrences are software-pipelined across chunks of T_c steps
    (layer l handles chunk sig-2l at super-step sig) so each layer's
    step-to-step DVE/ACT latency hides under the other layers' matmuls.
    W_hh stays pinned in SBUF (12MB f32); W_ih is streamed from DRAM.
    """
    nc = tc.nc
    B = cfg["B_loc"]; S = cfg["S"]; H = cfg["H"]; V = cfg["V"]
    L = cfg["L"]; T_c = cfg["T_c"]
    KC = H // P
    JC = KC
    NCH = S // T_c
    TOK = B * T_c
    TPT = P // B          # time steps per output token-tile
    assert S % T_c == 0 and H % P == 0 and TOK % P == 0

    emb_x = ins["emb_x"]      # [V, S*B] f32 (column s*B+b = emb[x[b,s],:])
    wih0 = ins["wih0"]        # [V, H]   f32 (w_ih0^T)
    wih = ins["wih"]          # [L-1, H, H] f32 (w_ih_hi[l]^T)
    whh = ins["whh"]          # [L, H, H]   f32 (w_hh[l]^T)
    wout = ins["wout"]        # [H, V]  f32 (w_out^T)
    bias = ins["bias"]        # [L, JC, P] f32 (b_ih+b_hh)
    bout = ins["bout"]        # [P, V]  f32 (b_out replicated)
    logprobs = outs["logprobs"]   # [S, B, V] f32
    hiddenT = outs["hiddenT"]     # [L, KC, P, B] f32

    def wtile(shape, dtype, nm):
        p = ctx.enter_context(tc.tile_pool(name=f"wp_{nm}", bufs=1))
        return p.tile(shape, dtype, tag=nm, name=nm)

    hpool = [ctx.enter_context(tc.tile_pool(name=f"h{l}", bufs=2)) for l in range(L)]
    prepool = [ctx.enter_context(tc.tile_pool(name=f"pre{l}", bufs=2)) for l in range(L)]
    wihpool = ctx.enter_context(tc.tile_pool(name="wihs", bufs=4))
    rec_psum = [
        ctx.enter_context(tc.tile_pool(name=f"rpsum{l}", bufs=1, space="PSUM"))
        for l in range(L)
    ]
    gemm_psum = ctx.enter_context(tc.tile_pool(name="gpsum", bufs=3, space="PSUM"))
    out_psum = ctx.enter_context(tc.tile_pool(name="opsum", bufs=2, space="PSUM"))
    spool = ctx.enter_context(tc.tile_pool(name="soft", bufs=3))
    hid_pool = ctx.enter_context(tc.tile_pool(name="hid", bufs=1))

    whh_sb = []
    for l in range(L):
        t = wtile([P, KC, H], F32, f"whh{l}")
        nc.sync.dma_start(t[:], whh[l].rearrange("(kc p) j -> p kc j", p=P))
        whh_sb.append(t)
    wih0_sb = wtile([V, H], F32, "wih0")
    nc.sync.dma_start(wih0_sb[:], wih0[:])
    wout_sb = wtile([P, KC, V], F32, "wout")
    nc.sync.dma_start(wout_sb[:], wout.rearrange("(kc p) v -> p kc v", p=P))
    emb_sb = wtile([V, S * B], F32, "embx")
    nc.sync.dma_start(emb_sb[:], emb_x[:])
    bias_sb = wtile([P, L, JC], F32, "bias")
    nc.sync.dma_start(bias_sb[:], bias.rearrange("l jc p -> p l jc"))
    bout_sb = wtile([P, V], F32, "bout")
    nc.sync.dma_start(bout_sb[:], bout[:])
    zero_sb = wtile([P, B], F32, "zero")
    nc.vector.memset(zero_sb[:], 0.0)

    h_bufs = [dict() for _ in range(L)]
    pre_bufs = [dict() for _ in range(L)]

    def pre0_stage(c):
        pre_t = prepool[0].tile([P, JC, T_c, B], F32, tag="pre0", name="pre0")
        pre_bufs[0][c] = pre_t
        rhs = emb_sb[:, c * TOK:(c + 1) * TOK]
        for jm in range(JC):
            ps = gemm_psum.tile([P, T_c, B], F32, tag="gemm", name="gps")
            nc.tensor.matmul(ps[:], wih0_sb[:, jm * P:(jm + 1) * P], rhs,
                             start=True, stop=True)
            nc.scalar.activation(pre_t[:, jm], ps[:], AF.Identity,
                                 bias=bias_sb[:, 0, jm:jm + 1])

    def gemm_stage(l, c):
        h_src = h_bufs[l - 1][c]
        pre_t = prepool[l].tile([P, JC, T_c, B], F32, tag=f"pre{l}", name="pre")
        pre_bufs[l][c] = pre_t
        wih_l = wih[l - 1].rearrange("(kc p) j -> p kc j", p=P)
        for jm in range(JC):
            wt = wihpool.tile([P, KC, P], F32, tag="wihs", name="wihs")
            nc.sync.dma_start(wt[:], wih_l[:, :, jm * P:(jm + 1) * P])
            ps = gemm_psum.tile([P, T_c, B], F32, tag="gemm", name="gps")
            for kc in range(KC):
                nc.tensor.matmul(ps[:], wt[:, kc, :], h_src[:, kc, :, :],
                                 start=(kc == 0), stop=(kc == KC - 1))
            nc.scalar.activation(pre_t[:, jm], ps[:], AF.Identity,
                                 bias=bias_sb[:, l, jm:jm + 1])

    def rec_step(l, c, t):
        tau = c * T_c + t
        ps = rec_psum[l].tile([P, JC, B], F32, tag=f"rec{l}", name="rps")
        if tau == 0:
            rhs = [zero_sb[:] for _ in range(KC)]
        elif t == 0:
            prev = h_bufs[l][c - 1]
            rhs = [prev[:, kc, T_c - 1, :] for kc in range(KC)]
        else:
            cur = h_bufs[l][c]
            rhs = [cur[:, kc, t - 1, :] for kc in range(KC)]
        for jm in range(JC):
            for kc in range(KC):
                nc.tensor.matmul(ps[:, jm], whh_sb[l][:, kc, jm * P:(jm + 1) * P],
                                 rhs[kc], start=(kc == 0), stop=(kc == KC - 1))
        pre_t = pre_bufs[l][c]
        nc.vector.scalar_tensor_tensor(ps[:], ps[:], 1.0, pre_t[:, :, t, :],
                                       op0=ALU.mult, op1=ALU.add)
        nc.scalar.activation(h_bufs[l][c][:, :, t, :], ps[:], AF.Tanh)
        if tau == S - 1:
            hid = hid_pool.tile([P, JC, B], F32, tag=f"hid{l}", name="hid")
            nc.scalar.activation(hid[:], ps[:], AF.Tanh)
            nc.sync.dma_start(hiddenT[l].rearrange("kc p b -> p kc b"), hid[:])

    def out_stage(c):
        h_src = h_bufs[L - 1][c]
        for it in range(TOK // P):
            t0 = it * TPT
            ps = out_psum.tile([P, V], F32, tag="out", name="ops")
            for kc in range(KC):
                nc.tensor.matmul(ps[:], h_src[:, kc, t0:t0 + TPT, :],
                                 wout_sb[:, kc, :],
                                 start=(kc == 0), stop=(kc == KC - 1))
            nc.vector.tensor_tensor(ps[:], ps[:], bout_sb[:], op=ALU.add)
            nmx = spool.tile([P, 1], F32, tag="nmx", name="nmx")
            nc.vector.tensor_reduce(nmx[:], ps[:], axis=mybir.AxisListType.X,
                                    op=ALU.max, negate=True)
            e = spool.tile([P, V], F32, tag="e", name="e")
            ssum = spool.tile([P, 1], F32, tag="ssum", name="ssum")
            nc.scalar.activation(e[:], ps[:], AF.Exp, bias=nmx[:], accum_out=ssum[:])
            lse = spool.tile([P, 1], F32, tag="lse", name="lse")
            nc.scalar.activation(lse[:], ssum[:], AF.Ln)
            off = spool.tile([P, 1], F32, tag="off", name="off")
            nc.vector.tensor_tensor(off[:], lse[:], nmx[:], op=ALU.subtract)
            res = spool.tile([P, V], F32, tag="res", name="res")
            nc.vector.tensor_scalar(res[:], ps[:], off[:], None, op0=ALU.subtract)
            s0 = c * T_c + t0
            nc.sync.dma_start(logprobs[s0:s0 + TPT], res[:])

    for sig in range(NCH + 2 * L):
        if sig < NCH:
            pre0_stage(sig)
        for l in range(L):
            c = sig - 2 * l
            if 0 <= c < NCH:
                h_bufs[l][c] = hpool[l].tile([P, KC, T_c, B], F32,
                                             tag=f"h{l}", name=f"h{l}")
        for t in range(T_c):
            for l in range(L):
                c = sig - 2 * l
                if 0 <= c < NCH:
                    rec_step(l, c, t)
        for l in range(1, L):
            c = sig - (2 * l - 1)
            if 0 <= c < NCH:
                gemm_stage(l, c)
        c = sig - (2 * L - 1)
        if 0 <= c < NCH:
            out_stage(c)


_NC_CACHE = None


def _get_program():
    global _NC_CACHE
    if _NC_CACHE is not None:
        return _NC_CACHE
    cfg = dict(B_loc=B_LOC, S=S, H=H, V=V, L=L, T_c=T_C)
    nc = bacc.Bacc("TRN2", target_bir_lowering=False, debug=False,
                   num_devices=N_CORES)
    ins = {
        "emb_x": nc.declare_dram_parameter("emb_x", [V, S * B_LOC], F32, isOutput=False),
        "wih0": nc.declare_dram_parameter("wih0", [V, H], F32, isOutput=False),
        "wih": nc.declare_dram_parameter("wih", [L - 1, H, H], F32, isOutput=False),
        "whh": nc.declare_dram_parameter("whh", [L, H, H], F32, isOutput=False),
        "wout": nc.declare_dram_parameter("wout", [H, V], F32, isOutput=False),
        "bias": nc.declare_dram_parameter("bias", [L, KC, P], F32, isOutput=False),
        "bout": nc.declare_dram_parameter("bout", [P, V], F32, isOutput=False),
    }
    outs = {
        "logprobs": nc.declare_dram_parameter("logprobs", [S, B_LOC, V], F32, isOutput=True),
        "hiddenT": nc.declare_dram_parameter("hiddenT", [L, KC, P, B_LOC], F32, isOutput=True),
    }
    with tile.TileContext(nc) as tc:
        with ExitStack() as ctx:
            build_charrnn(ctx, tc, outs, ins, cfg)
    nc.compile()
    _NC_CACHE = nc
    return nc


def _make_in_maps(x, emb, w_ih0, w_ih_hi, w_hh, b_ih, b_hh, w_out, b_out):
    shared = {
        "wih0": np.ascontiguousarray(w_ih0.T).astype(np.float32),
        "wih": np.ascontiguousarray(w_ih_hi.transpose(0, 2, 1)).astype(np.float32),
        "whh": np.ascontiguousarray(w_hh.transpose(0, 2, 1)).astype(np.float32),
        "wout": np.ascontiguousarray(w_out.T).astype(np.float32),
        "bias": (np.asarray(b_ih) + np.asarray(b_hh)).reshape(L, KC, P).astype(np.float32),
        "bout": np.ascontiguousarray(np.broadcast_to(b_out, (P, V))).astype(np.float32),
    }
    emb = np.asarray(emb, dtype=np.float32)
    x = np.asarray(x)
    in_maps = []
    for c in range(N_CORES):
        x_sh = x[c * B_LOC:(c + 1) * B_LOC]
        embx = emb[x_sh]                                   # [B_loc, S, V]
        emb_x = np.ascontiguousarray(
            embx.transpose(2, 1, 0).reshape(V, S * B_LOC)).astype(np.float32)
        in_maps.append({"emb_x": emb_x, **shared})
    return in_maps


def _assemble(core_results):
    lps, hids = [], []
    for r in core_results:
        lps.append(np.asarray(r["logprobs"]).transpose(1, 0, 2))       # [B_loc,S,V]
        ht = np.asarray(r["hiddenT"])                                   # [L,KC,P,B]
        hids.append(ht.transpose(0, 3, 1, 2).reshape(L, B_LOC, H))      # [L,B_loc,H]
    logprobs = np.concatenate(lps, axis=0).astype(np.float32)
    hidden = np.concatenate(hids, axis=1).astype(np.float32)
    return logprobs, hidden


def _run(in_maps, trace=False, **kw):
    nc = _get_program()
    return run_bass_kernel_spmd(nc, in_maps, list(range(N_CORES)), trace=trace, **kw)


def kernel(x, emb, w_ih0, w_ih_hi, w_hh, b_ih, b_hh, w_out, b_out):
    in_maps = _make_in_maps(x, emb, w_ih0, w_ih_hi, w_hh, b_ih, b_hh, w_out, b_out)
    res = _run(in_maps)
    return _assemble(res.results)


# revision 5
# speedup vs baseline: 1.1963x; 1.1963x over previous
"""CharRNN (B=128, S=256, V=96, H=1024, L=3) on 8 TRN2 NeuronCores.

Self-contained kernel: data-parallel over batch (16 per core), full fp32.

The recurrence h_t = tanh(pre_t + W_hh h_{t-1}) is chaotic (||W_hh|| ~ 3):
perturbations amplify ~10x per 32 steps, so sub-fp32 compute (bf16/fp16,
or flipping which matmul operand is stationary) drifts off the reference
trajectory and diverges by mid-sequence. This kernel keeps the exact
arithmetic shape the XLA/neuron reference uses - W-stationary fp32 matmuls
with K accumulated in 128-chunks in index order - which tracks the
reference to ~4e-5 absmax end-to-end. The cost is the fp32 weight-load
path (no fast-weight-load for fp32): the PE spends ~245ns per 128x128
weight-tile pass, which is the measured roofline of this design.

Per-core program (build_charrnn): feature-major layouts everywhere so the
recurrence needs no transposes:
  h buffers   [128(p=k%128), KC, T_c, B]  f32  (h[b, kc*128+p] at local t)
  pre buffers [128(p=j%128), JC, T_c, B]  f32
  weights     W^T tiles [128(k), kc, j]   f32  (lhsT stationary)
Per rec step: 64 matmuls (jm x kc), psum [128, JC, B] accumulates, DVE adds
the pre slice, ACT tanh writes the h slice. The 3 layer recurrences are
software-pipelined across chunks of T_c steps (layer l handles chunk
sig-2l at super-step sig) so each layer's DVE/ACT latency hides under the
other layers' matmuls. Inter-layer GEMMs, the output GEMM and log-softmax
run between chunks. W_hh stays pinned in SBUF (12MB); W_ih streams from
DRAM per (chunk, jm).
"""
from contextlib import ExitStack

import numpy as np

import concourse.bass as bass
import concourse.tile as tile
from concourse import bacc, mybir
from concourse.bass_utils import run_bass_kernel_spmd

F32 = mybir.dt.float32
AF = mybir.ActivationFunctionType
ALU = mybir.AluOpType
P = 128

N_CORES = 8
B_FULL, S, V, H, L = 128, 256, 96, 1024, 3
B_LOC = B_FULL // N_CORES
T_C = 8
KC = H // P


def build_charrnn(ctx: ExitStack, tc, outs: dict, ins: dict, cfg: dict):
    nc = tc.nc
    B = cfg["B_loc"]; S = cfg["S"]; H = cfg["H"]; V = cfg["V"]
    L = cfg["L"]; T_c = cfg["T_c"]
    KC = H // P
    JC = KC
    NCH = S // T_c
    TOK = B * T_c
    TPT = P // B          # time steps per output token-tile
    assert S % T_c == 0 and H % P == 0 and TOK % P == 0

    emb_x = ins["emb_x"]      # [V, S*B] f32 (column s*B+b = emb[x[b,s],:])
    wih0 = ins["wih0"]        # [V, H]   f32 (w_ih0^T)
    wih = ins["wih"]          # [L-1, H, H] f32 (w_ih_hi[l]^T)
    whh = ins["whh"]          # [L, H, H]   f32 (w_hh[l]^T)
    wout = ins["wout"]        # [H, V]  f32 (w_out^T)
    bias = ins["bias"]        # [L, JC, P] f32 (b_ih+b_hh)
    bout = ins["bout"]        # [P, V]  f32 (b_out replicated)
    logprobs = outs["logprobs"]   # [S, B, V] f32
    hiddenT = outs["hiddenT"]     # [L, KC, P, B] f32

    def wtile(shape, dtype, nm):
        p = ctx.enter_context(tc.tile_pool(name=f"wp_{nm}", bufs=1))
        return p.tile(shape, dtype, tag=nm, name=nm)

    hpool = [ctx.enter_context(tc.tile_pool(name=f"h{l}", bufs=2)) for l in range(L)]
    prepool = [ctx.enter_context(tc.tile_pool(name=f"pre{l}", bufs=2)) for l in range(L)]
    wihpool = ctx.enter_context(tc.tile_pool(name="wihs", bufs=4))
    rec_psum = [
        ctx.enter_context(tc.tile_pool(name=f"rpsum{l}", bufs=1, space="PSUM"))
        for l in range(L)
    ]
    gemm_psum = ctx.enter_context(tc.tile_pool(name="gpsum", bufs=3, space="PSUM"))
    out_psum = ctx.enter_context(tc.tile_pool(name="opsum", bufs=2, space="PSUM"))
    spool = ctx.enter_context(tc.tile_pool(name="soft", bufs=3))
    hid_pool = ctx.enter_context(tc.tile_pool(name="hid", bufs=1))

    whh_sb = []
    for l in range(L):
        t = wtile([P, KC, H], F32, f"whh{l}")
        nc.sync.dma_start(t[:], whh[l].rearrange("(kc p) j -> p kc j", p=P))
        whh_sb.append(t)
    wih0_sb = wtile([V, H], F32, "wih0")
    nc.sync.dma_start(wih0_sb[:], wih0[:])
    wout_sb = wtile([P, KC, V], F32, "wout")
    nc.sync.dma_start(wout_sb[:], wout.rearrange("(kc p) v -> p kc v", p=P))
    emb_sb = wtile([V, S * B], F32, "embx")
    nc.sync.dma_start(emb_sb[:], emb_x[:])
    bias_sb = wtile([P, L, JC], F32, "bias")
    nc.sync.dma_start(bias_sb[:], bias.rearrange("l jc p -> p l jc"))
    bout_sb = wtile([P, V], F32, "bout")
    nc.sync.dma_start(bout_sb[:], bout[:])
    zero_sb = wtile([P, B], F32, "zero")
    nc.vector.memset(zero_sb[:], 0.0)

    h_bufs = [dict() for _ in range(L)]
    pre_bufs = [dict() for _ in range(L)]

    def pre0_stage(c):
        pre_t = prepool[0].tile([P, JC, T_c, B], F32, tag="pre0", name="pre0")
        pre_bufs[0][c] = pre_t
        rhs = emb_sb[:, c * TOK:(c + 1) * TOK]
        for jm in range(JC):
            ps = gemm_psum.tile([P, T_c, B], F32, tag="gemm", name="gps")
            nc.tensor.matmul(ps[:], wih0_sb[:, jm * P:(jm + 1) * P], rhs,
                             start=True, stop=True)
            nc.scalar.activation(pre_t[:, jm], ps[:], AF.Identity,
                                 bias=bias_sb[:, 0, jm:jm + 1])

    def gemm_stage(l, c):
        h_src = h_bufs[l - 1][c]
        pre_t = prepool[l].tile([P, JC, T_c, B], F32, tag=f"pre{l}", name="pre")
        pre_bufs[l][c] = pre_t
        wih_l = wih[l - 1].rearrange("(kc p) j -> p kc j", p=P)
        for jm in range(JC):
            wt = wihpool.tile([P, KC, P], F32, tag="wihs", name="wihs")
            nc.sync.dma_start(wt[:], wih_l[:, :, jm * P:(jm + 1) * P])
            ps = gemm_psum.tile([P, T_c, B], F32, tag="gemm", name="gps")
            for kc in range(KC):
                nc.tensor.matmul(ps[:], wt[:, kc, :], h_src[:, kc, :, :],
                                 start=(kc == 0), stop=(kc == KC - 1))
            nc.scalar.activation(pre_t[:, jm], ps[:], AF.Identity,
                                 bias=bias_sb[:, l, jm:jm + 1])

    def rec_step(l, c, t):
        tau = c * T_c + t
        ps = rec_psum[l].tile([P, JC, B], F32, tag=f"rec{l}", name="rps")
        if tau == 0:
            rhs = [zero_sb[:] for _ in range(KC)]
        elif t == 0:
            prev = h_bufs[l][c - 1]
            rhs = [prev[:, kc, T_c - 1, :] for kc in range(KC)]
        else:
            cur = h_bufs[l][c]
            rhs = [cur[:, kc, t - 1, :] for kc in range(KC)]
        for jm in range(JC):
            for kc in range(KC):
                nc.tensor.matmul(ps[:, jm], whh_sb[l][:, kc, jm * P:(jm + 1) * P],
                                 rhs[kc], start=(kc == 0), stop=(kc == KC - 1))
        pre_t = pre_bufs[l][c]
        nc.vector.scalar_tensor_tensor(ps[:], ps[:], 1.0, pre_t[:, :, t, :],
                                       op0=ALU.mult, op1=ALU.add)
        nc.scalar.activation(h_bufs[l][c][:, :, t, :], ps[:], AF.Tanh)
        if tau == S - 1:
            hid = hid_pool.tile([P, JC, B], F32, tag=f"hid{l}", name="hid")
            nc.scalar.activation(hid[:], ps[:], AF.Tanh)
            nc.sync.dma_start(hiddenT[l].rearrange("kc p b -> p kc b"), hid[:])

    def out_stage(c):
        h_src = h_bufs[L - 1][c]
        for it in range(TOK // P):
            t0 = it * TPT
            ps = out_psum.tile([P, V], F32, tag="out", name="ops")
            for kc in range(KC):
                nc.tensor.matmul(ps[:], h_src[:, kc, t0:t0 + TPT, :],
                                 wout_sb[:, kc, :],
                                 start=(kc == 0), stop=(kc == KC - 1))
            nc.vector.tensor_tensor(ps[:], ps[:], bout_sb[:], op=ALU.add)
            nmx = spool.tile([P, 1], F32, tag="nmx", name="nmx")
            nc.vector.tensor_reduce(nmx[:], ps[:], axis=mybir.AxisListType.X,
                                    op=ALU.max, negate=True)
            e = spool.tile([P, V], F32, tag="e", name="e")
            ssum = spool.tile([P, 1], F32, tag="ssum", name="ssum")
            nc.scalar.activation(e[:], ps[:], AF.Exp, bias=nmx[:], accum_out=ssum[:])
            lse = spool.tile([P, 1], F32, tag="lse", name="lse")
            nc.scalar.activation(lse[:], ssum[:], AF.Ln)
            off = spool.tile([P, 1], F32, tag="off", name="off")
            nc.vector.tensor_tensor(off[:], lse[:], nmx[:], op=ALU.subtract)
            res = spool.tile([P, V], F32, tag="res", name="res")
            nc.vector.tensor_scalar(res[:], ps[:], off[:], None, op0=ALU.subtract)
            s0 = c * T_c + t0
            nc.sync.dma_start(logprobs[s0:s0 + TPT], res[:])

    for sig in range(NCH + 2 * L):
        if sig < NCH:
            pre0_stage(sig)
        for l in range(L):
            c = sig - 2 * l
            if 0 <= c < NCH:
                h_bufs[l][c] = hpool[l].tile([P, KC, T_c, B], F32,
                                             tag=f"h{l}", name=f"h{l}")
        for t in range(T_c):
            for l in range(L):
                c = sig - 2 * l
                if 0 <= c < NCH:
                    rec_step(l, c, t)
        for l in range(1, L):
            c = sig - (2 * l - 1)
            if 0 <= c < NCH:
                gemm_stage(l, c)
        c = sig - (2 * L - 1)
        if 0 <= c < NCH:
            out_stage(c)


_NC_CACHE = None


def _get_program():
    global _NC_CACHE
    if _NC_CACHE is not None:
        return _NC_CACHE
    cfg = dict(B_loc=B_LOC, S=S, H=H, V=V, L=L, T_c=T_C)
    nc = bacc.Bacc("TRN2", target_bir_lowering=False, debug=False,
                   num_devices=N_CORES)
    ins = {
        "emb_x": nc.declare_dram_parameter("emb_x", [V, S * B_LOC], F32, isOutput=False),
        "wih0": nc.declare_dram_parameter("wih0", [V, H], F32, isOutput=False),
        "wih": nc.declare_dram_parameter("wih", [L - 1, H, H], F32, isOutput=False),
        "whh": nc.declare_dram_parameter("whh", [L, H, H], F32, isOutput=False),
        "wout": nc.declare_dram_parameter("wout", [H, V], F32, isOutput=False),
        "bias": nc.declare_dram_parameter("bias", [L, KC, P], F32, isOutput=False),
        "bout": nc.declare_dram_parameter("bout", [P, V], F32, isOutput=False),
    }
    outs = {
        "logprobs": nc.declare_dram_parameter("logprobs", [S, B_LOC, V], F32, isOutput=True),
        "hiddenT": nc.declare_dram_parameter("hiddenT", [L, KC, P, B_LOC], F32, isOutput=True),
    }
    with tile.TileContext(nc) as tc:
        with ExitStack() as ctx:
            build_charrnn(ctx, tc, outs, ins, cfg)
    nc.compile()
    _NC_CACHE = nc
    return nc


def _make_in_maps(x, emb, w_ih0, w_ih_hi, w_hh, b_ih, b_hh, w_out, b_out):
    shared = {
        "wih0": np.ascontiguousarray(np.asarray(w_ih0).T).astype(np.float32),
        "wih": np.ascontiguousarray(np.asarray(w_ih_hi).transpose(0, 2, 1)).astype(np.float32),
        "whh": np.ascontiguousarray(np.asarray(w_hh).transpose(0, 2, 1)).astype(np.float32),
        "wout": np.ascontiguousarray(np.asarray(w_out).T).astype(np.float32),
        "bias": (np.asarray(b_ih) + np.asarray(b_hh)).reshape(L, KC, P).astype(np.float32),
        "bout": np.ascontiguousarray(np.broadcast_to(b_out, (P, V))).astype(np.float32),
    }
    emb = np.asarray(emb, dtype=np.float32)
    x = np.asarray(x)
    in_maps = []
    for c in range(N_CORES):
        x_sh = x[c * B_LOC:(c + 1) * B_LOC]
        embx = emb[x_sh]                                   # [B_loc, S, V]
        emb_x = np.ascontiguousarray(
            embx.transpose(2, 1, 0).reshape(V, S * B_LOC)).astype(np.float32)
        in_maps.append({"emb_x": emb_x, **shared})
    return in_maps


def _assemble(core_results):
    lps, hids = [], []
    for r in core_results:
        lps.append(np.asarray(r["logprobs"]).transpose(1, 0, 2))       # [B_loc,S,V]
        ht = np.asarray(r["hiddenT"])                                   # [L,KC,P,B]
        hids.append(ht.transpose(0, 3, 1, 2).reshape(L, B_LOC, H))      # [L,B_loc,H]
    logprobs = np.concatenate(lps, axis=0).astype(np.float32)
    hidden = np.concatenate(hids, axis=1).astype(np.float32)
    return logprobs, hidden


def _run(in_maps, trace=False, **kw):
    nc = _get_program()
    return run_bass_kernel_spmd(nc, in_maps, list(range(N_CORES)), trace=trace, **kw)


def kernel(x, emb, w_ih0, w_ih_hi, w_hh, b_ih, b_hh, w_out, b_out):
    in_maps = _make_in_maps(x, emb, w_ih0, w_ih_hi, w_hh, b_ih, b_hh, w_out, b_out)
    res = _run(in_maps)
    return _assemble(res.results)
